# revision 17
# baseline (speedup 1.0000x reference)
"""GNN message-passing layer (sum/max segment aggregation + 2-way attention
combine + 3-layer MLP) on 8 Trainium2 NeuronCores.

Strategy: nodes are sharded across the 8 cores (round-robin over a global
degree-sorted order so every core sees an identical degree profile). Each
core gathers its nodes' neighbor features straight from HBM with the custom
dma_gather (int16 indices => h is addressed as 25000 row-PAIRS; each node's
neighbor list is split by source-row parity and gathered from the even/odd
strided views of h), segment-reduces on the vector engine, and runs
attention+MLP on PE/ACT in a feature-major (transposed) layout. No
cross-core communication: every core gets full h and its own edge shard.
"""

import sys

sys.path.insert(0, "/opt/trn_rl_repo")

import numpy as np

import concourse.bass as bass
import concourse.bacc as bacc_mod
import concourse.tile as tile
from concourse import mybir
from concourse.bass_utils import run_bass_kernel_spmd
from concourse.tile import add_dep_helper

N = 50000
D = 64
NC = 8
P = 128
NPC = 6272  # padded nodes per core (49 tiles of 128)
T = NPC // P  # 49 tiles
KG = 64  # reduce-block slots (G tile size)
KCH = 8  # slots per dma_gather (1024 descriptors = ring capacity)
BIG = 1.0e4  # masks empty-parity max contributions
F32 = mybir.dt.float32
I16 = mybir.dt.int16

_prog_cache = {}


def _wrap16(A):
    """[P, K] int16 slot table -> [128, 8*K] dma_gather index layout.

    dma_gather reads flat index i from idx[i%16, i//16] (replicated across
    the 8 GPSIMD-core partition stripes) and writes slot (p, k) from flat
    i = k*128 + p.
    """
    Pp, K = A.shape
    assert Pp == P
    blk = A.reshape(8, 16, K).transpose(1, 2, 0).reshape(16, K * 8)
    return np.tile(blk, (8, 1))


# ---------------------------------------------------------------- host prep
def _prep(h, edge_index):
    ei = np.asarray(edge_index)
    rows = np.concatenate([ei[0], ei[1]]).astype(np.int64)
    cols = np.concatenate([ei[1], ei[0]]).astype(np.int64)
    deg = np.bincount(rows, minlength=N)

    order = np.argsort(-deg, kind="stable")  # nodes by degree, descending
    perm = np.argsort(rows, kind="stable")
    scols = cols[perm].astype(np.int32)

    # node lists per core (padded with sentinel -1)
    node_lists = []
    for c in range(NC):
        nl = order[c::NC]
        nl = np.concatenate([nl, -np.ones(NPC - len(nl), dtype=nl.dtype)])
        node_lists.append(nl)

    # split each node's CSR neighbor range by source parity (evens first)
    par = (scols & 1).astype(np.int64)
    seg_id = np.repeat(np.arange(N), deg)
    ne = np.bincount(seg_id[par == 0], minlength=N)
    no = np.bincount(seg_id[par == 1], minlength=N)
    order_eo = np.lexsort((par, seg_id))
    scols_eo = scols[order_eo]  # node 0 evens, node 0 odds, node 1 evens, ...
    pairidx = (scols_eo >> 1).astype(np.int16)
    indptr = np.concatenate([[0], np.cumsum(deg)]).astype(np.int64)
    e_start = indptr[:-1]
    o_start = indptr[:-1] + ne

    # per-tile K for each parity (max over the tile's global rank range)
    def tile_max(counts):
        Ks = []
        for t in range(T):
            r0, r1 = t * P * NC, min((t + 1) * P * NC, N)
            m = int(counts[order[r0:r1]].max()) if r0 < N else 0
            Ks.append(max(4, -(-m // 4) * 4))
        return Ks

    KE_list = tile_max(ne)
    KO_list = tile_max(no)

    def chunks(K):
        out = []
        k0 = 0
        while k0 < K:
            out.append(min(KG, K - k0))
            k0 += KG
        return out

    # chunk schedule: per tile, per parity, (col_offset, kc) pairs
    sched = []
    colE = colO = 0
    for t in range(T):
        ent = {"E": [], "O": []}
        for kc in chunks(KE_list[t]):
            ent["E"].append((colE, kc))
            colE += kc
        for kc in chunks(KO_list[t]):
            ent["O"].append((colO, kc))
            colO += kc
        sched.append(ent)
    SKE, SKO = colE, colO

    nbrE = np.zeros((NC, P, 8 * SKE), dtype=np.int16)
    nbrO = np.zeros((NC, P, 8 * SKO), dtype=np.int16)
    aux = np.zeros((NC, P, 5 * T), dtype=np.float32)
    hTown = np.zeros((NC, D, NPC), dtype=np.float32)
    hf = np.asarray(h, dtype=np.float32)

    Kmax = max(max(KE_list), max(KO_list))
    j = np.arange(Kmax)

    def padded_block(nodes, starts, counts, K):
        """[len(nodes), K] int16 pair-index table; dup-last padding; 0 for
        empty lists."""
        st = starts[nodes]
        cn = counts[nodes]
        cnc = np.maximum(cn, 1)
        gi = st[:, None] + np.minimum(j[None, :K], cnc[:, None] - 1)
        gi = np.minimum(gi, len(pairidx) - 1)
        blk = pairidx[gi].copy()
        blk[cn == 0] = 0
        return blk.astype(np.int16)

    for c in range(NC):
        nl = node_lists[c]
        real = nl >= 0
        nlr = np.where(real, nl, 0)
        hTown[c][:, real] = hf[nl[real]].T
        for t in range(T):
            sl = slice(t * P, (t + 1) * P)
            KE, KO = KE_list[t], KO_list[t]
            blkE = padded_block(nlr[sl], e_start, ne, KE)
            blkE[~real[sl]] = 0
            blkO = padded_block(nlr[sl], o_start, no, KO)
            blkO[~real[sl]] = 0
            k0 = 0
            for col, kc in sched[t]["E"]:
                nbrE[c][:, 8 * col : 8 * (col + kc)] = _wrap16(
                    np.ascontiguousarray(blkE[:, k0 : k0 + kc])
                )
                k0 += kc
            k0 = 0
            for col, kc in sched[t]["O"]:
                nbrO[c][:, 8 * col : 8 * (col + kc)] = _wrap16(
                    np.ascontiguousarray(blkO[:, k0 : k0 + kc])
                )
                k0 += kc
            eE = np.where(real[sl], ne[nlr[sl]], 0)
            eO = np.where(real[sl], no[nlr[sl]], 0)
            aux[c][:, 5 * t + 0] = (KE - eE).astype(np.float32)  # cntE
            aux[c][:, 5 * t + 1] = (KO - eO).astype(np.float32)  # cntO
            aux[c][:, 5 * t + 2] = np.where(eE > 0, 0.0, BIG)  # cBigE
            aux[c][:, 5 * t + 3] = np.where(eO > 0, 0.0, BIG)  # cBigO
            aux[c][:, 5 * t + 4] = ((eE + eO) > 0).astype(np.float32)  # flag

    meta = (tuple(KE_list), tuple(KO_list), sched, SKE, SKO)
    return meta, node_lists, nbrE, nbrO, aux, hTown


# ---------------------------------------------------------------- program
def _build_program(meta, debug_barrier=False, repeat=1):
    KE_list, KO_list, sched, SKE, SKO = meta
    key = (KE_list, KO_list, SKE, SKO, debug_barrier, repeat)
    if key in _prog_cache:
        return _prog_cache[key]

    nc = bacc_mod.Bacc(None, target_bir_lowering=False, num_swdge_queues=4)
    h_d = nc.declare_dram_parameter("h", [N, D], F32, isOutput=False)
    nbrE_d = nc.declare_dram_parameter("nbrE", [P, 8 * SKE], I16, isOutput=False)
    nbrO_d = nc.declare_dram_parameter("nbrO", [P, 8 * SKO], I16, isOutput=False)
    aux_d = nc.declare_dram_parameter("aux", [P, 5 * T], F32, isOutput=False)
    hT_d = nc.declare_dram_parameter("hT", [D, NPC], F32, isOutput=False)
    w1_d = nc.declare_dram_parameter("w1", [D, D], F32, isOutput=False)
    w2_d = nc.declare_dram_parameter("w2", [D, D], F32, isOutput=False)
    w3_d = nc.declare_dram_parameter("w3", [D, D], F32, isOutput=False)
    b1_d = nc.declare_dram_parameter("b1", [D, 1], F32, isOutput=False)
    b2_d = nc.declare_dram_parameter("b2", [D, 1], F32, isOutput=False)
    b3_d = nc.declare_dram_parameter("b3", [D, 1], F32, isOutput=False)
    wdiff_d = nc.declare_dram_parameter("wdiff", [P, 1], F32, isOutput=False)
    bdiff_d = nc.declare_dram_parameter("bdiff", [1, 2], F32, isOutput=False)
    ident_d = nc.declare_dram_parameter("ident", [P, P], F32, isOutput=False)
    ones_d = nc.declare_dram_parameter("ones", [1, D], F32, isOutput=False)
    out_d = nc.declare_dram_parameter("outT", [D, NPC], F32, isOutput=True)

    AF = mybir.ActivationFunctionType
    # h viewed as 25000 row pairs: [25000, 128]; even rows = cols 0:64
    hp = h_d[:, :].rearrange("(v two) d -> v (two d)", two=2)
    h_views = {"E": hp[:, 0:D], "O": hp[:, D : 2 * D]}

    with tile.TileContext(nc) as tc:
        with (
            tc.tile_pool(name="singles", bufs=1) as singles,
            tc.tile_pool(name="gp", bufs=4) as gp,
            tc.tile_pool(name="sp", bufs=3) as sp,
            tc.tile_pool(name="op", bufs=2) as op,
            tc.tile_pool(name="pp", bufs=2, space="PSUM") as pp,
        ):
            nbrE_sb = singles.tile([P, 8 * SKE], I16)
            nc.sync.dma_start(out=nbrE_sb[:], in_=nbrE_d[:, :])
            nbrO_sb = singles.tile([P, 8 * SKO], I16)
            nc.sync.dma_start(out=nbrO_sb[:], in_=nbrO_d[:, :])
            nbr_views = {"E": nbrE_sb, "O": nbrO_sb}
            aux_sb = singles.tile([P, 5 * T], F32)
            nc.sync.dma_start(out=aux_sb[:], in_=aux_d[:, :])
            hT_sb = singles.tile([D, NPC], F32)
            nc.sync.dma_start(out=hT_sb[:], in_=hT_d[:, :])
            w_sb = []
            for i, wd in enumerate((w1_d, w2_d, w3_d)):
                w = singles.tile([D, D], F32, tag=f"w{i}")
                nc.sync.dma_start(out=w[:], in_=wd[:, :])
                w_sb.append(w)
            b_sb = []
            for i, bd in enumerate((b1_d, b2_d, b3_d)):
                b = singles.tile([D, 1], F32, tag=f"b{i}")
                nc.sync.dma_start(out=b[:], in_=bd[:, :])
                b_sb.append(b)
            wdiff_sb = singles.tile([P, 1], F32)
            nc.sync.dma_start(out=wdiff_sb[:], in_=wdiff_d[:, :])
            bdiff_sb = singles.tile([1, 2], F32)
            nc.sync.dma_start(out=bdiff_sb[:], in_=bdiff_d[:, :])
            ident_sb = singles.tile([P, P], F32)
            nc.sync.dma_start(out=ident_sb[:], in_=ident_d[:, :])
            ones_sb = singles.tile([1, D], F32)
            nc.sync.dma_start(out=ones_sb[:], in_=ones_d[:, :])

            stage = None
            GBUFS = 4  # must match gp pool bufs
            # SWDGE pseudo-DMAs only support a single sync wait. Each
            # gather's natural deps are (a) WAW on the DMA that previously
            # filled its slot and (b) WAR on the DVE reduces that read it.
            # Keep (a) on the DMA; subsume (b) via a tiny Pool-engine memset
            # that explicitly waits on a finished tile's DVE readers,
            # advancing Pool's observed DVE clock before slots are reused.
            tile_of_chunk = {}
            pending_syncs = []  # (tile_idx, [DVE reader insts])
            gchunk = 0
            cur_tile = 0

            def flush_syncs(upto_tile):
                cps = []
                while pending_syncs and pending_syncs[0][0] <= upto_tile:
                    _, rds = pending_syncs.pop(0)
                    scratch = sp.tile([1, 1], F32, tag="gsync")
                    cp = nc.gpsimd.memset(scratch[:], 0.0)
                    for r in rds:
                        add_dep_helper(cp.ins, r.ins, sync=True)
                    cps.append(cp)
                return cps

            qn = [0]

            def gather(view, idx_sb, col, kg):
                """Fill a [P, kg, D] G tile with ceil(kg/KCH) dma_gathers."""
                nonlocal gchunk
                cps = flush_syncs(tile_of_chunk.get(gchunk - GBUFS, -1))
                G = gp.tile([P, kg, D], F32, tag="G")
                first = True
                for j in range(0, kg, KCH):
                    kc = min(KCH, kg - j)
                    gi = nc.gpsimd.dma_gather(
                        out_ap=G[:, j : j + kc, :],
                        in_ap=view,
                        idxs_ap=idx_sb[:, 8 * (col + j) : 8 * (col + j + kc)],
                        num_idxs=P * kc,
                        num_idxs_reg=P * kc,
                        elem_size=D,
                        elem_step=2 * D,
                        queue_num=qn[0],
                        single_packet=False,
                    )
                    qn[0] = (qn[0] + 1) % 4
                    if first:
                        for cp in cps:
                            add_dep_helper(gi.ins, cp.ins, sync=False)
                        first = False
                tile_of_chunk[gchunk] = cur_tile
                gchunk += 1
                return G

            for rep in range(repeat):
              for t in range(T):
                cur_tile = rep * T + t
                SM = sp.tile([P, 2, D], F32, tag="SM")
                readers = []

                # parity partial aggregates:
                #  E: sum into SM[:,0,:], max into SM[:,1,:]; O: SO / MO
                SO = sp.tile([P, D], F32, tag="SO")
                MO = sp.tile([P, D], F32, tag="MO")
                corrs = {}
                for pi, (pname, S_acc, M_acc) in enumerate(
                    (("E", SM[:, 0, :], SM[:, 1, :]), ("O", SO[:], MO[:]))
                ):
                    chs = sched[t][pname]
                    corr = sp.tile([P, D], F32, tag=f"corr{pname}")
                    corrs[pname] = corr
                    for ci, (col, kg) in enumerate(chs):
                        G = gather(h_views[pname], nbr_views[pname], col, kg)
                        g3 = G[:, :, :].rearrange("p k d -> p d k")
                        if ci == len(chs) - 1:
                            # dup-padding corr: cnt * (last slot = dup nbr)
                            readers.append(
                                nc.vector.tensor_scalar_mul(
                                    corr[:],
                                    G[:, kg - 1, :],
                                    aux_sb[:, 5 * t + pi : 5 * t + pi + 1],
                                )
                            )
                        if ci == 0:
                            readers.append(
                                nc.vector.tensor_reduce(
                                    out=S_acc,
                                    in_=g3,
                                    axis=mybir.AxisListType.X,
                                    op=mybir.AluOpType.add,
                                )
                            )
                            readers.append(
                                nc.vector.tensor_reduce(
                                    out=M_acc,
                                    in_=g3,
                                    axis=mybir.AxisListType.X,
                                    op=mybir.AluOpType.max,
                                )
                            )
                        else:
                            Sp = sp.tile([P, D], F32, tag="Sp")
                            Mp = sp.tile([P, D], F32, tag="Mp")
                            readers.append(
                                nc.vector.tensor_reduce(
                                    out=Sp[:],
                                    in_=g3,
                                    axis=mybir.AxisListType.X,
                                    op=mybir.AluOpType.add,
                                )
                            )
                            readers.append(
                                nc.vector.tensor_reduce(
                                    out=Mp[:],
                                    in_=g3,
                                    axis=mybir.AxisListType.X,
                                    op=mybir.AluOpType.max,
                                )
                            )
                            nc.vector.tensor_add(S_acc, S_acc, Sp[:])
                            nc.vector.tensor_tensor(
                                out=M_acc, in0=M_acc, in1=Mp[:],
                                op=mybir.AluOpType.max,
                            )
                pending_syncs.append((cur_tile, readers))

                S = SM[:, 0, :]
                M = SM[:, 1, :]
                # S = SE + SO - corrE - corrO
                nc.vector.tensor_add(S, S, SO[:])
                nc.vector.tensor_sub(S, S, corrs["E"][:])
                nc.vector.tensor_sub(S, S, corrs["O"][:])
                # M = flag * max(ME - cBigE, MO - cBigO)
                nc.vector.tensor_scalar_sub(M, M, aux_sb[:, 5 * t + 2 : 5 * t + 3])
                nc.vector.tensor_scalar_sub(
                    MO[:], MO[:], aux_sb[:, 5 * t + 3 : 5 * t + 4]
                )
                nc.vector.tensor_tensor(
                    out=M, in0=M, in1=MO[:], op=mybir.AluOpType.max
                )
                nc.vector.tensor_scalar_mul(M, M, aux_sb[:, 5 * t + 4 : 5 * t + 5])
                # transpose [sum|max] -> catT rows: 0-63 sumT, 64-127 maxT
                catP = pp.tile([P, P], F32, tag="catP")
                nc.tensor.transpose(
                    catP[:], SM[:, :, :].rearrange("p a d -> p (a d)"), ident_sb[:]
                )
                cat = sp.tile([P, P], F32, tag="cat")
                nc.scalar.copy(cat[:], catP[:])
                # attention: d01 = (attw[:,0]-attw[:,1]).T @ cat -> [1,128]
                d01 = pp.tile([1, P], F32, tag="d01")
                nc.tensor.matmul(
                    d01[:], lhsT=wdiff_sb[:], rhs=cat[:], start=True, stop=True
                )
                wts = sp.tile([1, 2 * P], F32, tag="wts")
                nc.scalar.activation(
                    wts[:, 0:P], d01[:], AF.Sigmoid, bias=bdiff_sb[:, 0:1]
                )
                nc.scalar.activation(
                    wts[:, P : 2 * P],
                    d01[:],
                    AF.Sigmoid,
                    bias=bdiff_sb[:, 1:2],
                    scale=-1.0,
                )
                wb = pp.tile([D, 2 * P], F32, tag="wb")
                nc.tensor.matmul(
                    wb[:], lhsT=ones_sb[:], rhs=wts[:], start=True, stop=True
                )
                xT = sp.tile([D, P], F32, tag="xT")
                tmp = sp.tile([D, P], F32, tag="tmp")
                nc.vector.tensor_mul(xT[:], cat[0:D, :], wb[:, 0:P])
                nc.vector.tensor_mul(tmp[:], cat[D:P, :], wb[:, P : 2 * P])
                nc.vector.tensor_add(xT[:], xT[:], tmp[:])
                nc.vector.tensor_add(xT[:], xT[:], hT_sb[:, t * P : (t + 1) * P])
                # MLP
                m1 = pp.tile([D, P], F32, tag="mm")
                nc.tensor.matmul(m1[:], lhsT=w_sb[0][:], rhs=xT[:], start=True, stop=True)
                a1 = sp.tile([D, P], F32, tag="a1")
                nc.scalar.activation(a1[:], m1[:], AF.Relu, bias=b_sb[0][:])
                m2 = pp.tile([D, P], F32, tag="mm")
                nc.tensor.matmul(m2[:], lhsT=w_sb[1][:], rhs=a1[:], start=True, stop=True)
                a2 = sp.tile([D, P], F32, tag="a2")
                nc.scalar.activation(a2[:], m2[:], AF.Relu, bias=b_sb[1][:])
                m3 = pp.tile([D, P], F32, tag="mm")
                nc.tensor.matmul(m3[:], lhsT=w_sb[2][:], rhs=a2[:], start=True, stop=True)
                s_off = (t % 8) * P
                if s_off == 0:
                    stage = op.tile([D, 8 * P], F32, tag="stage")
                nc.scalar.activation(
                    stage[:, s_off : s_off + P], m3[:], AF.Identity, bias=b_sb[2][:]
                )
                if t % 8 == 7 or t == T - 1:
                    base = (t // 8) * 8 * P
                    nc.sync.dma_start(
                        out=out_d[:, base : base + s_off + P],
                        in_=stage[:, 0 : s_off + P],
                    )
                if debug_barrier:
                    tc.strict_bb_all_engine_barrier()

    nc.compile()
    _prog_cache[key] = nc
    return nc


# ---------------------------------------------------------------- driver
def run(inputs, trace=False, debug_barrier=False):
    h = np.ascontiguousarray(np.asarray(inputs["h"], dtype=np.float32))
    meta, node_lists, nbrE, nbrO, aux, hTown = _prep(h, inputs["edge_index"])
    nc = _build_program(meta, debug_barrier=debug_barrier)

    att_w = np.asarray(inputs["att_w"], dtype=np.float32)
    att_b = np.asarray(inputs["att_b"], dtype=np.float32)
    wdiff = (att_w[:, 0] - att_w[:, 1]).reshape(P, 1).astype(np.float32)
    bd = float(att_b[0] - att_b[1])
    bdiff = np.array([[bd, -bd]], dtype=np.float32)
    shared = {
        "h": h,
        "w1": np.asarray(inputs["w1"], dtype=np.float32),
        "w2": np.asarray(inputs["w2"], dtype=np.float32),
        "w3": np.asarray(inputs["w3"], dtype=np.float32),
        "b1": np.asarray(inputs["b1"], dtype=np.float32).reshape(D, 1),
        "b2": np.asarray(inputs["b2"], dtype=np.float32).reshape(D, 1),
        "b3": np.asarray(inputs["b3"], dtype=np.float32).reshape(D, 1),
        "wdiff": wdiff,
        "bdiff": bdiff,
        "ident": np.eye(P, dtype=np.float32),
        "ones": np.ones((1, D), dtype=np.float32),
    }
    in_maps = []
    for c in range(NC):
        m = dict(shared)
        m["nbrE"] = nbrE[c]
        m["nbrO"] = nbrO[c]
        m["aux"] = aux[c]
        m["hT"] = hTown[c]
        in_maps.append(m)

    res = run_bass_kernel_spmd(nc, in_maps, list(range(NC)), trace=trace)

    out = np.zeros((N, D), dtype=np.float32)
    for c in range(NC):
        nl = node_lists[c]
        real = nl >= 0
        out[nl[real]] = res.results[c]["outT"].T[real]
    return out, res.exec_time_ns


def _make_in_maps(inputs, prep):
    meta, node_lists, nbrE, nbrO, aux, hTown = prep
    att_w = np.asarray(inputs["att_w"], dtype=np.float32)
    att_b = np.asarray(inputs["att_b"], dtype=np.float32)
    wdiff = (att_w[:, 0] - att_w[:, 1]).reshape(P, 1).astype(np.float32)
    bd = float(att_b[0] - att_b[1])
    h = np.ascontiguousarray(np.asarray(inputs["h"], dtype=np.float32))
    shared = {
        "h": h,
        "w1": np.asarray(inputs["w1"], dtype=np.float32),
        "w2": np.asarray(inputs["w2"], dtype=np.float32),
        "w3": np.asarray(inputs["w3"], dtype=np.float32),
        "b1": np.asarray(inputs["b1"], dtype=np.float32).reshape(D, 1),
        "b2": np.asarray(inputs["b2"], dtype=np.float32).reshape(D, 1),
        "b3": np.asarray(inputs["b3"], dtype=np.float32).reshape(D, 1),
        "wdiff": wdiff,
        "bdiff": np.array([[bd, -bd]], dtype=np.float32),
        "ident": np.eye(P, dtype=np.float32),
        "ones": np.ones((1, D), dtype=np.float32),
    }
    in_maps = []
    for c in range(NC):
        m = dict(shared)
        m["nbrE"] = nbrE[c]
        m["nbrO"] = nbrO[c]
        m["aux"] = aux[c]
        m["hT"] = hTown[c]
        in_maps.append(m)
    return in_maps


def time_kernel(inputs, R=9, iters=12):
    """Estimate HW exec time via repeat-amplified wall-clock deltas."""
    import bench

    h = np.ascontiguousarray(np.asarray(inputs["h"], dtype=np.float32))
    prep = _prep(h, inputs["edge_index"])
    in_maps = _make_in_maps(inputs, prep)
    nc1 = _build_program(prep[0], repeat=1)
    ncR = _build_program(prep[0], repeat=R)
    _, t1, t1med = bench.time_program(nc1, in_maps, NC, warmup=3, iters=iters)
    _, tR, tRmed = bench.time_program(ncR, in_maps, NC, warmup=3, iters=iters)
    exec_ns = (tRmed - t1med) / (R - 1) * 1e9
    return exec_ns, t1med, tRmed


def kernel(**inputs) -> np.ndarray:
    out, _ = run(inputs)
    return out
